# revision 4
# baseline (speedup 1.0000x reference)
"""Causal multi-head attention (B=4, S=2048, D=2048, H=16) on 8 TRN2 NeuronCores.

Sharding: core c = 2*b + g handles batch b (of 4) and head-group g (of 2,
8 heads each).  Megatron-style: q/k/v projections are column-parallel over
the head dimension, the output projection is row-parallel; the host sums
the two partial outputs per batch and adds the bias.

All matmuls run as fp32r (TF32-like full-rate mode on the PE). Softmax
skips the max-subtraction (scores are ~N(0,1); exp cannot overflow) so the
whole attention needs no partition-dim reductions: scores are computed
transposed [sk, sq], the softmax denominator comes from a ones-vector
matmul, and normalization is deferred to after attn@v.
"""

import math

import numpy as np

B, S, D = 4, 2048, 2048
H_TOTAL, DH = 16, 128
G = 2               # tensor-parallel head groups
HG = H_TOTAL // G   # 8 heads per group
F = HG * DH         # 1024 features per group
N_CORES = 8

_CACHE = {}


def _build_nc(iters=1):
    import concourse.mybir as mybir
    from concourse import bacc
    from concourse.tile import TileContext
    from concourse.masks import make_upper_triangular

    FP32R = mybir.dt.float32r
    F32 = mybir.dt.float32
    AF = mybir.ActivationFunctionType
    MUL = mybir.AluOpType.mult

    DT = D // 128    # 16 contraction tiles
    ST = S // 128    # 16 seq tiles
    FT = F // 128    # 8 feature tiles (= heads per group)
    SB = S // 512    # 4 seq blocks
    FB = F // 512    # 2 feature blocks

    nc = bacc.Bacc("TRN2", target_bir_lowering=False, debug=False)
    xT = nc.dram_tensor("xT", [D, S], FP32R, kind="ExternalInput")
    wq = nc.dram_tensor("wq", [D, F], FP32R, kind="ExternalInput")
    wk = nc.dram_tensor("wk", [D, F], FP32R, kind="ExternalInput")
    wv = nc.dram_tensor("wv", [D, F], FP32R, kind="ExternalInput")
    wo = nc.dram_tensor("wo", [F, D], FP32R, kind="ExternalInput")
    out = nc.dram_tensor("partial", [S, D], F32, kind="ExternalOutput")
    qT_s = nc.dram_tensor("qT_s", [F, S], FP32R)
    kT_s = nc.dram_tensor("kT_s", [F, S], FP32R)
    v_s = nc.dram_tensor("v_s", [S, F], FP32R)
    oT_s = nc.dram_tensor("oT_s", [F, S], FP32R)

    with TileContext(nc) as tc:
        with tc.tile_pool(name="const", bufs=1) as cp:
            # [tri | ones]: mask[:, 0:512-c0] masks a diagonal block at
            # column c0 of a 512-wide score block.
            m32 = cp.tile([128, 512], F32)
            make_upper_triangular(nc, m32[:, 0:128], val=1.0, diag=True)
            nc.gpsimd.memset(m32[:, 128:512], 1.0)
            mask = cp.tile([128, 512], FP32R)
            nc.vector.tensor_copy(mask[:], m32[:])
            o32 = cp.tile([128, 1], F32)
            nc.gpsimd.memset(o32[:], 1.0)
            ones = cp.tile([128, 1], FP32R)
            nc.vector.tensor_copy(ones[:], o32[:])

            for _ in range(iters):
                # ---- phase 1: q/k/v projections -------------------------
                with (
                    tc.tile_pool(name="ph1", bufs=1) as p1,
                    tc.tile_pool(name="ps1", bufs=1, space="PSUM") as ps1,
                ):
                    xt = p1.tile([128, DT, S], FP32R)  # x.T fully resident
                    for d in range(DT):
                        nc.sync.dma_start(
                            out=xt[:, d, :], in_=xT[d * 128 : (d + 1) * 128, :]
                        )

                    for f in range(FT):
                        wqf = p1.tile([128, DT, 128], FP32R, tag="wqf", bufs=2)
                        wkf = p1.tile([128, DT, 128], FP32R, tag="wkf", bufs=2)
                        fs = slice(f * 128, (f + 1) * 128)
                        nc.sync.dma_start(
                            out=wqf[:], in_=wq[:, fs].rearrange("(t p) f -> p t f", p=128)
                        )
                        nc.sync.dma_start(
                            out=wkf[:], in_=wk[:, fs].rearrange("(t p) f -> p t f", p=128)
                        )
                        for sb in range(SB):
                            ss = slice(sb * 512, (sb + 1) * 512)
                            for w_t, dst in ((wqf, qT_s), (wkf, kT_s)):
                                acc = ps1.tile([128, 512], F32, tag="ps_qk", bufs=2)
                                for d in range(DT):
                                    nc.tensor.matmul(
                                        acc[:],
                                        w_t[:, d, :],
                                        xt[:, d, ss],
                                        start=(d == 0),
                                        stop=(d == DT - 1),
                                    )
                                ev = p1.tile([128, 512], FP32R, tag="ev_qk", bufs=2)
                                nc.vector.tensor_copy(ev[:], acc[:])
                                nc.sync.dma_start(out=dst[fs, ss], in_=ev[:])

                    for fb in range(FB):
                        wvb = p1.tile([128, DT, 512], FP32R, tag="wvb", bufs=1)
                        fbs = slice(fb * 512, (fb + 1) * 512)
                        nc.sync.dma_start(
                            out=wvb[:], in_=wv[:, fbs].rearrange("(t p) f -> p t f", p=128)
                        )
                        for st in range(ST):
                            acc = ps1.tile([128, 512], F32, tag="ps_v", bufs=2)
                            for d in range(DT):
                                nc.tensor.matmul(
                                    acc[:],
                                    xt[:, d, st * 128 : (st + 1) * 128],
                                    wvb[:, d, :],
                                    start=(d == 0),
                                    stop=(d == DT - 1),
                                )
                            ev = p1.tile([128, 512], FP32R, tag="ev_v", bufs=2)
                            nc.vector.tensor_copy(ev[:], acc[:])
                            nc.sync.dma_start(
                                out=v_s[st * 128 : (st + 1) * 128, fbs], in_=ev[:]
                            )

                # ---- phase 2: causal attention per head -----------------
                with (
                    tc.tile_pool(name="ph2", bufs=1) as p2,
                    tc.tile_pool(name="ps2s", bufs=1, space="PSUM") as ps2s,
                    tc.tile_pool(name="ps2o", bufs=1, space="PSUM") as ps2o,
                ):
                    for h in range(HG):
                        hs = slice(h * 128, (h + 1) * 128)
                        qth = p2.tile([128, S], FP32R, tag="qth", bufs=2)
                        kth = p2.tile([128, S], FP32R, tag="kth", bufs=2)
                        vh = p2.tile([128, ST, DH], FP32R, tag="vh", bufs=2)
                        nc.sync.dma_start(out=qth[:], in_=qT_s[hs, :])
                        nc.sync.dma_start(out=kth[:], in_=kT_s[hs, :])
                        nc.sync.dma_start(
                            out=vh[:], in_=v_s[:, hs].rearrange("(t p) f -> p t f", p=128)
                        )
                        for b in range(SB):
                            bs = slice(b * 512, (b + 1) * 512)
                            acc_o = ps2o.tile([128, 512], F32, tag="ps_o", bufs=2)
                            acc_l = ps2o.tile([1, 512], F32, tag="ps_l", bufs=2)
                            jmax = 4 * b + 3
                            for j in range(jmax + 1):
                                sc = ps2s.tile([128, 512], F32, tag="ps_s", bufs=3)
                                nc.tensor.matmul(
                                    sc[:],
                                    kth[:, j * 128 : (j + 1) * 128],
                                    qth[:, bs],
                                    start=True,
                                    stop=True,
                                )
                                pt = p2.tile([128, 512], FP32R, tag="pt", bufs=4)
                                a = j - 4 * b
                                if a < 0:
                                    nc.scalar.activation(pt[:], sc[:], AF.Exp)
                                else:
                                    c0 = a * 128
                                    if c0:
                                        nc.vector.memset(
                                            pt[:, 0:c0].bitcast(F32), 0.0
                                        )
                                    nc.scalar.activation(
                                        pt[:, c0:512], sc[:, c0:512], AF.Exp
                                    )
                                    nc.vector.tensor_tensor(
                                        out=pt[:, c0:512],
                                        in0=pt[:, c0:512],
                                        in1=mask[:, 0 : 512 - c0],
                                        op=MUL,
                                    )
                                nc.tensor.matmul(
                                    acc_o[:],
                                    vh[:, j, :],
                                    pt[:],
                                    start=(j == 0),
                                    stop=(j == jmax),
                                )
                                nc.tensor.matmul(
                                    acc_l[:],
                                    ones[:],
                                    pt[:],
                                    start=(j == 0),
                                    stop=(j == jmax),
                                )
                            linv = p2.tile([1, 512], F32, tag="linv", bufs=2)
                            nc.vector.reciprocal(linv[:], acc_l[:])
                            linb = p2.tile([128, 512], F32, tag="linb", bufs=2)
                            nc.gpsimd.partition_broadcast(linb[:], linv[:])
                            otb = p2.tile([128, 512], FP32R, tag="otb", bufs=3)
                            nc.vector.tensor_tensor(
                                out=otb[:], in0=acc_o[:], in1=linb[:], op=MUL
                            )
                            nc.sync.dma_start(out=oT_s[hs, bs], in_=otb[:])

                # ---- phase 3: output projection -------------------------
                with (
                    tc.tile_pool(name="ph3", bufs=1) as p3,
                    tc.tile_pool(name="ps3", bufs=1, space="PSUM") as ps3,
                ):
                    wof = p3.tile([128, FT, D], FP32R)
                    nc.sync.dma_start(
                        out=wof[:], in_=wo.rearrange("(t p) f -> p t f", p=128)
                    )
                    for st in range(ST):
                        sts = slice(st * 128, (st + 1) * 128)
                        ot = p3.tile([128, FT, 128], FP32R, tag="ot", bufs=2)
                        nc.sync.dma_start(
                            out=ot[:],
                            in_=oT_s[:, sts].rearrange("(t p) s -> p t s", p=128),
                        )
                        for ob in range(SB):
                            obs = slice(ob * 512, (ob + 1) * 512)
                            acc = ps3.tile([128, 512], F32, tag="ps_p", bufs=2)
                            for f in range(FT):
                                nc.tensor.matmul(
                                    acc[:],
                                    ot[:, f, :],
                                    wof[:, f, obs],
                                    start=(f == 0),
                                    stop=(f == FT - 1),
                                )
                            po = p3.tile([128, 512], F32, tag="po", bufs=4)
                            nc.vector.tensor_copy(po[:], acc[:])
                            nc.sync.dma_start(out=out[sts, obs], in_=po[:])

    nc.compile()
    return nc


def _get_nc(iters=1):
    key = ("nc", iters)
    if key not in _CACHE:
        _CACHE[key] = _build_nc(iters)
    return _CACHE[key]


def make_in_maps(x, Wq, Wk, Wv, Wo):
    scale = 1.0 / math.sqrt(DH)
    xTs = [np.ascontiguousarray(x[b].T) for b in range(B)]
    in_maps = []
    for c in range(N_CORES):
        b, g = divmod(c, G)
        gs = slice(g * F, (g + 1) * F)
        in_maps.append(
            {
                "xT": xTs[b],
                "wq": np.ascontiguousarray(Wq[gs, :].T) * np.float32(scale),
                "wk": np.ascontiguousarray(Wk[gs, :].T),
                "wv": np.ascontiguousarray(Wv[gs, :].T),
                "wo": np.ascontiguousarray(Wo[:, gs].T),
            }
        )
    return in_maps


def kernel(x, Wq, Wk, Wv, Wo, bo):
    from concourse.bass_utils import run_bass_kernel_spmd

    x = np.asarray(x, dtype=np.float32)
    Wq = np.asarray(Wq, dtype=np.float32)
    Wk = np.asarray(Wk, dtype=np.float32)
    Wv = np.asarray(Wv, dtype=np.float32)
    Wo = np.asarray(Wo, dtype=np.float32)
    bo = np.asarray(bo, dtype=np.float32)

    nc = _get_nc()
    in_maps = make_in_maps(x, Wq, Wk, Wv, Wo)
    res = run_bass_kernel_spmd(nc, in_maps, list(range(N_CORES)))
    out = np.empty((B, S, D), dtype=np.float32)
    for b in range(B):
        out[b] = res.results[2 * b]["partial"] + res.results[2 * b + 1]["partial"] + bo
    return out
